# revision 1
# baseline (speedup 1.0000x reference)
"""Complex self-attention (single-head) on 8 Trainium2 NeuronCores.

Problem: y = stack(re, im) of softmax(|q k^H|/sqrt(D)) @ v with complex
q/k/v projections of a complex input x.  B=8, N=1024, D=512, fp32 I/O.

Strategy
--------
Data-parallel over the batch: core c computes batch c entirely locally.

Per-core math (all matmuls fp16 operands, fp32 PSUM accumulation):
  * Host pre-transposes x (-> x^T [D, N]) and ships transposed / negated /
    pre-scaled weight variants, so no on-device transposes are needed.
  * sqrt(1/sqrt(D)) is folded into BOTH Wq and Wk (and bq, bk) so the
    score scale comes out exactly right with zero device work.
  * Projections accumulate complex parts directly in PSUM:
      q^T = Wq^T.T @ x^T   (lhsT = Wq^T chunk, rhs = x^T chunk)
      v   = x^T.T @ Wv^T   (lhsT = x^T chunk, rhs = Wv^T chunk)
  * Scores are computed TRANSPOSED, s^T[m, n] = sum_e k^T[e,m] q^T[e,n],
    so that E = exp(|s|) lands in [m, n] layout, which is exactly the
    lhsT layout the att@v matmul wants.  Softmax then needs NO max, NO
    transpose and NO vector reductions:
      - exp without max-subtraction is safe (|s| <= ~20); a constant
        EXP_SHIFT keeps exp() within fp16 range, and cancels in U/Z.
      - Z[n] = sum_m E[m,n] comes from a matmul against a ones column.
      - w = (E^T.T @ v) * (1/Z) with a per-partition scalar multiply.
"""

from contextlib import ExitStack

import numpy as np

import concourse.bass as bass
import concourse.mybir as mybir
import concourse.tile as tile
from concourse import bacc
from concourse.bass_utils import run_bass_kernel_spmd

B, N, D = 8, 1024, 512
P = 128
KC = D // P          # 4 contraction chunks of 128
MC = N // P          # 8 row chunks of 128
NH = 2               # halves of N (free dim <= 512 per matmul)
NCORES = 8
EXP_SHIFT = 11.0     # exp(|s| - SHIFT): keeps E in fp16 range; cancels in U/Z

f16 = mybir.dt.float16
f32 = mybir.dt.float32
AF = mybir.ActivationFunctionType


def emit(tc, ctx, nc, xr_d, xi_d, w_d, bqk_d, out_d):
    singles = ctx.enter_context(tc.tile_pool(name="singles", bufs=1))
    ps = ctx.enter_context(tc.tile_pool(name="ps", bufs=7, space="PSUM"))
    psz = ctx.enter_context(tc.tile_pool(name="psz", bufs=1, space="PSUM"))
    tmp = ctx.enter_context(tc.tile_pool(name="tmp", bufs=4))
    outp = ctx.enter_context(tc.tile_pool(name="outp", bufs=4))

    # ---- inputs to SBUF -------------------------------------------------
    # All loads on the SP HWDGE ring (FIFO), split per contraction-chunk and
    # interleaved in exact first-use order so matmuls start early.
    xr_sb = singles.tile([P, KC, N], f16)
    xi_sb = singles.tile([P, KC, N], f16)
    xs_sb = singles.tile([P, KC, N], f16)    # x_re + x_im (computed on DVE)
    xr_r = xr_d.rearrange("(c p) n -> p c n", p=P)
    xi_r = xi_d.rearrange("(c p) n -> p c n", p=P)
    w_sb = singles.tile([P, 9, KC, D], f16)
    w_r = [w_d[t].rearrange("(c p) e -> p c e", p=P) for t in range(9)]

    bqk_sb = singles.tile([P, 4, KC], f32)
    # single HWDGE ring (SP): interleave weight/x chunks in exact first-use
    # order so the projection matmuls are never waiting on a later transfer.
    for kc in range(KC):
        nc.sync.dma_start(out=w_sb[:, 0, kc], in_=w_r[0][:, kc])
        nc.sync.dma_start(out=xr_sb[:, kc], in_=xr_r[:, kc])
        if kc == 0:  # tiny; after the critical first pair
            nc.sync.dma_start(out=bqk_sb, in_=bqk_d)
    for kc in range(KC):
        nc.sync.dma_start(out=w_sb[:, 1, kc], in_=w_r[1][:, kc])
        nc.sync.dma_start(out=xi_sb[:, kc], in_=xi_r[:, kc])
        nc.vector.tensor_add(xs_sb[:, kc], xr_sb[:, kc], xi_sb[:, kc])
    for t in (2, 3, 4, 5, 6, 7, 8):
        for kc in range(KC):
            nc.sync.dma_start(out=w_sb[:, t, kc], in_=w_r[t][:, kc])

    ones_m = singles.tile([P, 1], f16)
    nc.vector.memset(ones_m, 1.0)
    shift_sb = singles.tile([P, 1], f32)
    nc.vector.memset(shift_sb, -EXP_SHIFT)

    # ---- persistent intermediates --------------------------------------
    # qk_sb slots: 0 qr^T, 1 qi^T, 2 -qi^T, 3 kr^T, 4 ki^T   (each [e, n])
    qk_sb = singles.tile([P, 5, KC, N], f16)
    v_sb = singles.tile([P, 2, MC, D], f16)      # v[, m-chunk, d] re/im
    et_sb = singles.tile([P, MC, N], f16)        # E^T[m, n] = exp(|s|-SHIFT)

    # ---- q/k projections (Karatsuba: 3 products per complex matmul) -----
    # T1 = Wr x_re, T2 = Wi x_im, T3 = Wsum x_sum;
    # re = T1 - T2 + b_r;  im = T3 - T1 - T2 + b_i.
    # (w slots r/i/sum, bias slots b_r / b_r+b_i, dst slots re/im/neg-im)
    qk_spec = [
        ((0, 1, 2), (0, 1), (0, 1, 2)),      # q (also writes -qi)
        ((3, 4, 5), (2, 3), (3, 4, None)),   # k
    ]
    t1bp = ctx.enter_context(tc.tile_pool(name="t1bp", bufs=9))
    t12p = ctx.enter_context(tc.tile_pool(name="t12p", bufs=9))
    tiles = [(ec, nh) for ec in range(KC) for nh in range(NH)]
    for (w_r, w_i, w_s), (b_r, b_s), (d_r, d_i, d_n) in qk_spec:
        # phase A: T1 = Wr x_re -> t1b = T1 + b_r   (only needs Wr + x_re).
        # For the very first pair, run kc-major over groups of 4 tiles so PE
        # consumes (w chunk, x chunk) pairs in DMA arrival order.
        t1bs, t12s = {}, {}
        for g0 in range(0, len(tiles), 4):
            grp = tiles[g0:g0 + 4]
            pts = [ps.tile([P, 512], f32, tag="b", name="pt") for _ in grp]
            for kc in range(KC):
                for pt, (ec, nh) in zip(pts, grp):
                    nc.tensor.matmul(
                        pt, lhsT=w_sb[:, w_r, kc, ec * P:(ec + 1) * P],
                        rhs=xr_sb[:, kc, nh * 512:nh * 512 + 512],
                        start=(kc == 0), stop=(kc == KC - 1),
                    )
            for pt, (ec, nh) in zip(pts, grp):
                t1b = t1bp.tile([P, 512], f32, tag="t1b", name="t1b")
                nc.scalar.activation(
                    out=t1b, in_=pt, func=AF.Identity,
                    bias=bqk_sb[:, b_r, ec:ec + 1],
                )
                t1bs[ec, nh] = t1b
        # phase B: T2 = Wi x_im -> re = t1b - T2 ; t12b = T2 + t1b
        for ec, nh in tiles:
            n0, e0 = nh * 512, ec * P
            pt = ps.tile([P, 512], f32, tag="b", name="pt")
            for kc in range(KC):
                nc.tensor.matmul(
                    pt, lhsT=w_sb[:, w_i, kc, e0:e0 + P],
                    rhs=xi_sb[:, kc, n0:n0 + 512],
                    start=(kc == 0), stop=(kc == KC - 1),
                )
            nc.vector.scalar_tensor_tensor(
                out=qk_sb[:, d_r, ec, n0:n0 + 512],
                in0=pt, scalar=-1.0, in1=t1bs[ec, nh],
                op0=mybir.AluOpType.mult, op1=mybir.AluOpType.add,
            )
            t12b = t12p.tile([P, 512], f32, tag="t12b", name="t12b")
            nc.vector.tensor_add(t12b, pt, t1bs[ec, nh])
            t12s[ec, nh] = t12b
        # phase C: T3 = Wsum x_sum -> im = (T3 + b_sum) - t12b  (and -im)
        for ec, nh in tiles:
            n0, e0 = nh * 512, ec * P
            pt = ps.tile([P, 512], f32, tag="b", name="pt")
            for kc in range(KC):
                nc.tensor.matmul(
                    pt, lhsT=w_sb[:, w_s, kc, e0:e0 + P],
                    rhs=xs_sb[:, kc, n0:n0 + 512],
                    start=(kc == 0), stop=(kc == KC - 1),
                )
            nc.vector.scalar_tensor_tensor(
                out=qk_sb[:, d_i, ec, n0:n0 + 512],
                in0=pt, scalar=bqk_sb[:, b_s, ec:ec + 1], in1=t12s[ec, nh],
                op0=mybir.AluOpType.add, op1=mybir.AluOpType.subtract,
            )
            if d_n is not None:  # -qi from qi (fast fp16 sbuf pass)
                nc.vector.tensor_scalar(
                    qk_sb[:, d_n, ec, n0:n0 + 512],
                    qk_sb[:, d_i, ec, n0:n0 + 512],
                    -1.0, None, mybir.AluOpType.mult,
                )

    # ---- v projection (Karatsuba; bias deferred to after attention -------
    # since softmax rows sum to 1, w = U/Z + bv exactly) ------------------
    for mc in range(MC):
        m0 = mc * P
        prods = []
        for xs, wi in ((xr_sb, 6), (xi_sb, 7), (xs_sb, 8)):
            pt = ps.tile([P, 512], f32, tag="b", name="pt")
            for kc in range(KC):
                nc.tensor.matmul(
                    pt,
                    lhsT=xs[:, kc, m0:m0 + P],
                    rhs=w_sb[:, wi, kc, :],
                    start=(kc == 0),
                    stop=(kc == KC - 1),
                )
            prods.append(pt)
        t1, t2, t3 = prods
        t1s = t1bp.tile([P, 512], f32, tag="t1b", name="t1s")
        nc.scalar.activation(out=t1s, in_=t1, func=AF.Copy)
        nc.vector.scalar_tensor_tensor(
            out=v_sb[:, 0, mc, :], in0=t2, scalar=-1.0, in1=t1s,
            op0=mybir.AluOpType.mult, op1=mybir.AluOpType.add,
        )
        t12 = t12p.tile([P, 512], f32, tag="t12b", name="t12")
        nc.vector.tensor_add(t12, t2, t1s)
        nc.vector.scalar_tensor_tensor(
            out=v_sb[:, 1, mc, :], in0=t3, scalar=0.0, in1=t12,
            op0=mybir.AluOpType.bypass, op1=mybir.AluOpType.subtract,
        )

    # ---- scores + softmax numerator / AV, half by half ------------------
    def scores_half(nh):
        n0 = nh * 512
        for mc in range(MC):
            m0 = mc * P
            rt = ps.tile([P, 512], f32, tag="b", name="rt")
            it = ps.tile([P, 512], f32, tag="b", name="it")
            for out_t, pairs in ((rt, ((3, 0), (4, 2))), (it, ((3, 1), (4, 0)))):
                idx = 0
                for kt, qt in pairs:
                    for ec in range(KC):
                        nc.tensor.matmul(
                            out_t,
                            lhsT=qk_sb[:, kt, ec, m0:m0 + P],
                            rhs=qk_sb[:, qt, ec, n0:n0 + 512],
                            start=(idx == 0),
                            stop=(idx == 7),
                        )
                        idx += 1
            t1 = tmp.tile([P, 512], f32, tag="sq", name="t1")
            nc.scalar.activation(out=t1, in_=rt, func=AF.Square)
            t2 = tmp.tile([P, 512], f32, tag="sq", name="t2")
            nc.scalar.activation(out=t2, in_=it, func=AF.Square)
            u = tmp.tile([P, 512], f32, tag="u", name="u")
            nc.vector.tensor_add(u, t1, t2)
            a = tmp.tile([P, 512], f32, tag="a", name="a")
            nc.scalar.activation(out=a, in_=u, func=AF.Sqrt)
            nc.scalar.activation(
                out=et_sb[:, mc, n0:n0 + 512], in_=a, func=AF.Exp,
                bias=shift_sb,
            )

    def av_half(nh):
        for g in range(nh * 4, nh * 4 + 4):
            last = g == 7
            zp = psz.tile([P, 1], f32, tag="z", name="zp")
            if last:  # Z first so 1/Z is ready while U is still accumulating
                for mc in range(MC):
                    nc.tensor.matmul(
                        zp, lhsT=et_sb[:, mc, g * P:(g + 1) * P], rhs=ones_m,
                        start=mc == 0, stop=mc == MC - 1,
                    )
                zr = tmp.tile([P, 1], f32, tag="zr", name="zr")
                nc.vector.reciprocal(zr, zp)
            for h0, hw in ((0, 512),):
                ur = ps.tile([P, 512], f32, tag="b", name="ur")
                ui = ps.tile([P, 512], f32, tag="b", name="ui")
                for mc in range(MC):
                    lh = et_sb[:, mc, g * P:(g + 1) * P]
                    st, sp = mc == 0, mc == MC - 1
                    nc.tensor.matmul(ur[:, :hw], lhsT=lh,
                                     rhs=v_sb[:, 0, mc, h0:h0 + hw], start=st, stop=sp)
                    nc.tensor.matmul(ui[:, :hw], lhsT=lh,
                                     rhs=v_sb[:, 1, mc, h0:h0 + hw], start=st, stop=sp)
                    if not last:
                        nc.tensor.matmul(zp, lhsT=lh, rhs=ones_m, start=st, stop=sp)
                if not last:
                    zr = tmp.tile([P, 1], f32, tag="zr", name="zr")
                    nc.vector.reciprocal(zr, zp)
                # w = U * (1/Z); the v bias is added on the host (exact,
                # since sum(att) = 1). re on DVE, im on ACT: the two chains
                # of the final chunk run in parallel at the tail.
                o0 = outp.tile([P, 512], f16, tag="o", name="o0")
                nc.vector.tensor_scalar_mul(o0[:, :hw], ur[:, :hw], zr)
                nc.sync.dma_start(
                    out=out_d[0, g * P:(g + 1) * P, h0:h0 + hw], in_=o0[:, :hw])
                o1 = outp.tile([P, 512], f16, tag="o", name="o1")
                nc.scalar.activation(out=o1[:, :hw], in_=ui[:, :hw],
                                     func=AF.Copy, scale=zr)
                nc.scalar.dma_start(
                    out=out_d[1, g * P:(g + 1) * P, h0:h0 + hw], in_=o1[:, :hw])

    scores_half(0)
    av_half(0)
    scores_half(1)
    av_half(1)


def build_nc():
    nc = bacc.Bacc("TRN2", target_bir_lowering=False, debug=False)
    xr_d = nc.dram_tensor("xrT", [D, N], f16, kind="ExternalInput").ap()
    xi_d = nc.dram_tensor("xiT", [D, N], f16, kind="ExternalInput").ap()
    w_d = nc.dram_tensor("w9", [9, D, D], f16, kind="ExternalInput").ap()
    bqk_d = nc.dram_tensor("bqk", [P, 4, KC], f32, kind="ExternalInput").ap()
    out_d = nc.dram_tensor("out", [2, N, D], f16, kind="ExternalOutput").ap()
    with tile.TileContext(nc) as tc, ExitStack() as ctx:
        emit(tc, ctx, nc, xr_d, xi_d, w_d, bqk_d, out_d)
    nc.compile()
    return nc


def make_in_maps(inputs):
    sc = float((1.0 / np.sqrt(D)) ** 0.5)

    def t16(a, s=1.0):
        return np.ascontiguousarray(a.T * s).astype(np.float16)

    # w slots: q r/i/sum (scaled), k r/i/sum (scaled), v r / i / -i
    w9 = np.stack([
        t16(inputs["Wq_re"], sc), t16(inputs["Wq_im"], sc),
        t16(inputs["Wq_re"] + inputs["Wq_im"], sc),
        t16(inputs["Wk_re"], sc), t16(inputs["Wk_im"], sc),
        t16(inputs["Wk_re"] + inputs["Wk_im"], sc),
        t16(inputs["Wv_re"]), t16(inputs["Wv_im"]),
        t16(inputs["Wv_re"] + inputs["Wv_im"]),
    ])
    bqk = np.stack([
        inputs["bq_re"] * sc, (inputs["bq_re"] + inputs["bq_im"]) * sc,
        inputs["bk_re"] * sc, (inputs["bk_re"] + inputs["bk_im"]) * sc,
    ]).astype(np.float32)                       # [4, 512]
    bqk = bqk.reshape(4, KC, P).transpose(2, 0, 1).copy()  # [128, 4, KC]

    xrT = inputs["x_re"].transpose(0, 2, 1).astype(np.float16)  # [B, D, N]
    xiT = inputs["x_im"].transpose(0, 2, 1).astype(np.float16)
    return [
        {
            "xrT": np.ascontiguousarray(xrT[c]),
            "xiT": np.ascontiguousarray(xiT[c]),
            "w9": w9,
            "bqk": bqk,
        }
        for c in range(NCORES)
    ]


_NC_CACHE = None


def get_nc():
    global _NC_CACHE
    if _NC_CACHE is None:
        _NC_CACHE = build_nc()
    return _NC_CACHE


def kernel(**inputs) -> np.ndarray:
    nc = get_nc()
    in_maps = make_in_maps(inputs)
    res = run_bass_kernel_spmd(nc, in_maps, core_ids=list(range(NCORES)))
    out = np.stack([res.results[c]["out"] for c in range(NCORES)], axis=1)
    out = out.astype(np.float32)
    out[0] += inputs["bv_re"].astype(np.float32)
    out[1] += inputs["bv_im"].astype(np.float32)
    return out



# revision 36
# speedup vs baseline: 1.1038x; 1.1038x over previous
"""Complex self-attention (single-head) on 8 Trainium2 NeuronCores.

Problem: y = stack(re, im) of softmax(|q k^T|/sqrt(D)) @ v with complex
q/k/v projections of a complex input x.  B=8, N=1024, D=512, fp32 I/O.

Strategy
--------
Data-parallel over the batch: core c computes batch c entirely locally.

Key algebraic restructure vs the naive pipeline: the scores only need
q k^T = (x Wq^T + bq)(x Wk^T + bk)^T, so the two projections fold into
ONE matrix on the host:

    s = x A x^T + uq[n] + uk[m] + c0,   A = (Wq^T Wk) * scale

with uq = x (Wq^T bk) * scale + c0, uk = x (Wk^T bq) * scale computed on
the HOST (they only cost O(N D) there).  Device work per core is then

    Y^T = A^T x^T          (complex, Karatsuba: 3 matmuls)   96 mm
    v   = x Wv^T           (complex, Karatsuba: 3 matmuls)   96 mm
    s^T = x^T . Y^T        (complex, Karatsuba: 3 products) 192 mm
          + 2 rank-1 corrections per tile (ones x uq rows)   32 mm
    w   = att @ v          (real x complex: 2 matmuls)      128 mm

down from 672 f=512 matmuls in the naive version to 544.  uk enters via
the per-partition bias slot of the Activation engine; uq via rank-1 PE
accumulations (lhsT = ones row).

Softmax needs NO max pass: |s| <= ~20 so exp(|s| - SHIFT) is fp16-safe
and the shift cancels in U/Z.  Z comes from a ones-column matmul.

ACT table discipline: Identity/Copy/Square/Sqrt all live in the
`sqrt_and_others` table and Exp in `exp_and_others`; per score half we
emit [e1, sq, sq, sqrt] x 8 then [exp] x 8 so only 2 table loads per
half instead of 2 per tile.
"""

from contextlib import ExitStack

import numpy as np

import concourse.bass as bass
import concourse.mybir as mybir
import concourse.tile as tile
from concourse import bacc
from concourse.bass_utils import run_bass_kernel_spmd

B, N, D = 8, 1024, 512
P = 128
KC = D // P          # 4 contraction chunks of 128
MC = N // P          # 8 row chunks of 128
NH = 2               # halves of N (free dim <= 512 per matmul)
NCORES = 8
EXP_SHIFT = 11.0     # exp(|s| - SHIFT): keeps E in fp16 range; cancels in U/Z

f16 = mybir.dt.float16
f32 = mybir.dt.float32
AF = mybir.ActivationFunctionType
ALU = mybir.AluOpType


NL_EXP_TABLE = 6  # act_func_sets index of natural_log_exp_and_others


def emit(tc, ctx, nc, xr_d, xi_d, a_d, uq_d, uk_d, out_d):
    # Pre-load the one activation table that covers every func we use
    # (ln/exp/square/copy/identity); the compile pass then inserts no others.
    nc.scalar.add_instruction(mybir.InstLoadActFuncSet(
        name=nc.get_next_instruction_name(), ins=[], outs=[],
        act_func_set_id=NL_EXP_TABLE,
    ))
    singles = ctx.enter_context(tc.tile_pool(name="singles", bufs=1))
    ps = ctx.enter_context(tc.tile_pool(name="ps", bufs=7, space="PSUM"))
    psz = ctx.enter_context(tc.tile_pool(name="psz", bufs=1, space="PSUM"))
    t1bp = ctx.enter_context(tc.tile_pool(name="t1bp", bufs=9))
    t12p = ctx.enter_context(tc.tile_pool(name="t12p", bufs=9))
    tmp = ctx.enter_context(tc.tile_pool(name="tmp", bufs=4))
    outp = ctx.enter_context(tc.tile_pool(name="outp", bufs=4))

    # ---- inputs to SBUF -------------------------------------------------
    xr_sb = singles.tile([P, KC, N], f16)
    xi_sb = singles.tile([P, KC, N], f16)
    xs_sb = singles.tile([P, KC, N], f16)    # x_re + x_im (computed on DVE)
    a_sb = singles.tile([P, 6, KC, D], f16)  # Ar Ai As WvrT WviT WvsT
    uq_sb = singles.tile([1, 2, N], f16)     # rows: uq_re, uq_re+uq_im
    uk_sb = singles.tile([P, MC, 2], f32)    # cols: uk_re, uk_re+uk_im

    xr_r = xr_d.rearrange("(c p) n -> p c n", p=P)
    xi_r = xi_d.rearrange("(c p) n -> p c n", p=P)
    a_r = [a_d[t].rearrange("(c p) e -> p c e", p=P) for t in range(6)]

    # Medium-grain DMA in first-use order: fine first chunks so phase A
    # starts early, coarse tails to amortize the shared-DMA-device cost.
    # A-matrix chunks on the SP ring, x chunks on the (idle) DVE ring, so
    # the first lhsT/rhs pair arrives in parallel rather than serially.
    nc.sync.dma_start(out=a_sb[:, 0], in_=a_r[0])                    # Ar
    nc.scalar.dma_start(out=xr_sb[:, :, 0:512], in_=xr_r[:, :, 0:512])
    nc.sync.dma_start(out=a_sb[:, 1], in_=a_r[1])                    # Ai
    nc.scalar.dma_start(out=xr_sb[:, :, 512:1024], in_=xr_r[:, :, 512:1024])
    nc.scalar.dma_start(out=xi_sb[:, :, 0:512], in_=xi_r[:, :, 0:512])
    nc.sync.dma_start(out=a_sb[:, 2], in_=a_r[2])                    # As
    nc.scalar.dma_start(out=xi_sb[:, :, 512:1024], in_=xi_r[:, :, 512:1024])
    nc.sync.dma_start(out=uq_sb, in_=uq_d)
    nc.sync.dma_start(out=uk_sb, in_=uk_d)
    for t in (3, 4, 5):                                              # Wv
        nc.sync.dma_start(out=a_sb[:, t], in_=a_r[t])

    # Warm-up: dummy matmuls on a memset tile keep the PE busy (and its
    # p-state ramp running) while the first input DMAs land — the first
    # real matmul then issues back-to-back at full clock.
    dum_sb = singles.tile([1, 512], f16)
    nc.gpsimd.memset(dum_sb, 0.0)
    dum_ps = psz.tile([1, 512], f32, tag="z", name="dum_ps")
    for _ in range(19):
        nc.tensor.matmul(dum_ps, lhsT=dum_sb[0:1, 0:1], rhs=dum_sb,
                         start=True, stop=True)

    # xs = xr + xi on the (otherwise idle) Pool engine, per kc x half
    for kc in range(KC):
        for h0 in (0, 512):
            nc.gpsimd.tensor_add(
                xs_sb[:, kc, h0:h0 + 512], xr_sb[:, kc, h0:h0 + 512],
                xi_sb[:, kc, h0:h0 + 512])

    ones_m = singles.tile([P, 1], f16)
    nc.gpsimd.memset(ones_m, 1.0)
    ones_row = singles.tile([1, P], f16)
    nc.gpsimd.memset(ones_row, 1.0)
    shift_sb = singles.tile([P, 1], f32)
    nc.gpsimd.memset(shift_sb, -EXP_SHIFT)

    # ---- persistent intermediates --------------------------------------
    y_sb = singles.tile([P, 3, KC, N], f16)      # Yr^T / Yi^T / Ys^T  [e, n]
    v_sb = singles.tile([P, 2, MC, D], f16)      # v[, m-chunk, d] re/im
    et_sb = singles.tile([P, MC, N], f16)        # E^T[m, n] = exp(|s|-SHIFT)

    # ---- Y projection: Y^T = A^T x^T (Karatsuba) ------------------------
    # T1 = Ar^T xr^T ; T2 = Ai^T xi^T ; T3 = As^T xs^T
    # Yr = T1 - T2 ; Yi = T3 - T1 - T2 ; Ys = Yr + Yi
    tiles = [(ec, nh) for nh in range(NH) for ec in range(KC)]
    t1bs, t12s = {}, {}
    # phase A: groups of 4, kc-major so PE consumes chunks in DMA order
    for g0 in range(0, len(tiles), 4):
        grp = tiles[g0:g0 + 4]
        pts = [ps.tile([P, 512], f32, tag="b", name="pt") for _ in grp]
        for kc in range(KC):
            for pt, (ec, nh) in zip(pts, grp):
                nc.tensor.matmul(
                    pt, lhsT=a_sb[:, 0, kc, ec * P:(ec + 1) * P],
                    rhs=xr_sb[:, kc, nh * 512:nh * 512 + 512],
                    start=(kc == 0), stop=(kc == KC - 1),
                )
        for pt, (ec, nh) in zip(pts, grp):
            t1b = t1bp.tile([P, 512], f32, tag="t1b", name="t1b")
            nc.scalar.activation(out=t1b, in_=pt, func=AF.Copy)
            t1bs[ec, nh] = t1b
    # phase B
    for ec, nh in tiles:
        n0 = nh * 512
        pt = ps.tile([P, 512], f32, tag="b", name="pt")
        for kc in range(KC):
            nc.tensor.matmul(
                pt, lhsT=a_sb[:, 1, kc, ec * P:(ec + 1) * P],
                rhs=xi_sb[:, kc, n0:n0 + 512],
                start=(kc == 0), stop=(kc == KC - 1),
            )
        nc.vector.scalar_tensor_tensor(
            out=y_sb[:, 0, ec, n0:n0 + 512],
            in0=pt, scalar=-1.0, in1=t1bs[ec, nh],
            op0=ALU.mult, op1=ALU.add,
        )
        t12 = t12p.tile([P, 512], f32, tag="t12", name="t12")
        nc.vector.tensor_add(t12, pt, t1bs[ec, nh])
        t12s[ec, nh] = t12
    # phase C
    for ec, nh in tiles:
        n0 = nh * 512
        pt = ps.tile([P, 512], f32, tag="b", name="pt")
        for kc in range(KC):
            nc.tensor.matmul(
                pt, lhsT=a_sb[:, 2, kc, ec * P:(ec + 1) * P],
                rhs=xs_sb[:, kc, n0:n0 + 512],
                start=(kc == 0), stop=(kc == KC - 1),
            )
        nc.vector.tensor_sub(y_sb[:, 1, ec, n0:n0 + 512], pt, t12s[ec, nh])
        nc.vector.tensor_add(                       # fp16 2x mode
            y_sb[:, 2, ec, n0:n0 + 512],
            y_sb[:, 0, ec, n0:n0 + 512], y_sb[:, 1, ec, n0:n0 + 512],
        )

    # ---- v projection (Karatsuba; bias deferred to the host: since the
    # softmax rows sum to 1, w = U/Z + bv exactly).  Emitted AFTER the
    # first score half so its matmuls fill PE bubbles there. ---------------
    def v_tile(mc):
        # Two PSUM banks per tile: T3 accumulates on top of T1 in bank A
        # after t1s snapshots it.  vr = t1s - T2;
        # vi = T3 - T1 - T2 = (bankA = T1+T3) - (2*t1s + T2).
        m0 = mc * P
        bank_a = ps.tile([P, 512], f32, tag="b", name="bank_a")
        for kc in range(KC):
            nc.tensor.matmul(
                bank_a, lhsT=xr_sb[:, kc, m0:m0 + P], rhs=a_sb[:, 3, kc, :],
                start=(kc == 0), stop=(kc == KC - 1),
            )
        t1s = t1bp.tile([P, 512], f32, tag="t1b", name="t1s")
        nc.scalar.activation(out=t1s, in_=bank_a, func=AF.Copy)
        p2 = ps.tile([P, 512], f32, tag="b", name="p2v")
        for kc in range(KC):
            nc.tensor.matmul(
                p2, lhsT=xi_sb[:, kc, m0:m0 + P], rhs=a_sb[:, 4, kc, :],
                start=(kc == 0), stop=(kc == KC - 1),
            )
        for kc in range(KC):
            nc.tensor.matmul(
                bank_a, lhsT=xs_sb[:, kc, m0:m0 + P], rhs=a_sb[:, 5, kc, :],
                start=False, stop=(kc == KC - 1), skip_group_check=True,
            )
        nc.vector.scalar_tensor_tensor(
            out=v_sb[:, 0, mc, :], in0=p2, scalar=-1.0, in1=t1s,
            op0=ALU.mult, op1=ALU.add,
        )
        t12 = t12p.tile([P, 512], f32, tag="t12", name="t12v")
        nc.vector.scalar_tensor_tensor(
            out=t12, in0=t1s, scalar=2.0, in1=p2,
            op0=ALU.mult, op1=ALU.add,
        )
        nc.vector.tensor_sub(v_sb[:, 1, mc, :], bank_a, t12)

    # ---- scores + softmax numerator / AV --------------------------------
    def score_tile(nh, mc):
        if True:
            n0 = nh * 512
            m0 = mc * P
            # Bank A holds P1 = xr.Yr + uq_re (rank-1); e1 = P1 + uk_re
            # snapshots it, then P3 = xs.Ys + uq_sum ACCUMULATES on top, so
            # each score tile needs only TWO psum banks:
            #   sr = e1 - P2
            #   si = P3 - P1 - P2 + uk_im
            #      = (bankA + (2*uk_re + uk_im)) - (2*e1 + P2)
            bank_a = ps.tile([P, 512], f32, tag="b", name="bank_a")
            nc.tensor.matmul(bank_a, lhsT=ones_row,
                             rhs=uq_sb[0:1, 0, n0:n0 + 512],
                             start=True, stop=False)
            for ec in range(KC):
                nc.tensor.matmul(
                    bank_a, lhsT=xr_sb[:, ec, m0:m0 + P],
                    rhs=y_sb[:, 0, ec, n0:n0 + 512],
                    start=False, stop=(ec == KC - 1),
                )
            e1 = t1bp.tile([P, 512], f32, tag="t1b", name="e1")
            nc.scalar.activation(out=e1, in_=bank_a, func=AF.Identity,
                                 bias=uk_sb[:, mc, 0:1])
            p2 = ps.tile([P, 512], f32, tag="b", name="p2")
            for ec in range(KC):
                nc.tensor.matmul(
                    p2, lhsT=xi_sb[:, ec, m0:m0 + P],
                    rhs=y_sb[:, 1, ec, n0:n0 + 512],
                    start=(ec == 0), stop=(ec == KC - 1),
                )
            # stop on the P1 group above is sim bookkeeping only — on HW the
            # bank simply keeps accumulating (start=False), giving P1 + P3.
            nc.tensor.matmul(bank_a, lhsT=ones_row,
                             rhs=uq_sb[0:1, 1, n0:n0 + 512],
                             start=False, stop=False, skip_group_check=True)
            for ec in range(KC):
                nc.tensor.matmul(
                    bank_a, lhsT=xs_sb[:, ec, m0:m0 + P],
                    rhs=y_sb[:, 2, ec, n0:n0 + 512],
                    start=False, stop=(ec == KC - 1), skip_group_check=True,
                )
            sr = tmp.tile([P, 512], f32, tag="s", name="sr")
            nc.vector.scalar_tensor_tensor(
                out=sr, in0=p2, scalar=-1.0, in1=e1,
                op0=ALU.mult, op1=ALU.add,
            )
            t12 = t12p.tile([P, 512], f32, tag="t12", name="t12s")
            nc.vector.scalar_tensor_tensor(
                out=t12, in0=e1, scalar=2.0, in1=p2,
                op0=ALU.mult, op1=ALU.add,
            )
            si = tmp.tile([P, 512], f32, tag="s", name="si")
            nc.vector.scalar_tensor_tensor(
                out=si, in0=bank_a, scalar=uk_sb[:, mc, 1:2], in1=t12,
                op0=ALU.add, op1=ALU.subtract,
            )
            t1 = tmp.tile([P, 512], f32, tag="sq", name="t1")
            nc.gpsimd.tensor_mul(t1, sr, sr)
            t2 = tmp.tile([P, 512], f32, tag="sq", name="t2")
            nc.gpsimd.tensor_mul(t2, si, si)
            u = tmp.tile([P, 512], f32, tag="u", name="u")
            nc.vector.tensor_add(u, t1, t2)
            # |s| = sqrt(u) = exp(0.5 ln u): Ln/Exp/Square/Copy/Identity all
            # share ONE activation table (natural_log_exp_and_others), so the
            # whole kernel needs a single table load — sqrt would thrash.
            lnu = tmp.tile([P, 512], f32, tag="u", name="lnu")
            nc.scalar.activation(out=lnu, in_=u, func=AF.Ln)
            sabs = tmp.tile([P, 512], f32, tag="sq", name="sabs")
            nc.scalar.activation(out=sabs, in_=lnu, func=AF.Exp, scale=0.5)
            nc.scalar.activation(
                out=et_sb[:, mc, n0:n0 + 512], in_=sabs, func=AF.Exp,
                bias=shift_sb,
            )

    av_zr = {}

    def av_half(nh, defer_last_group):
        """AV for one half with the last m-chunk of each group DEFERRED:
        groups g0..g2 accumulate chunks 0..6 first (plain PE work with no
        fresh dependencies), so the last score tile's DVE/Pool/ACT chain
        drains while PE stays busy; the mc=7 chunks + outputs come last.
        The 4th group can be deferred entirely (emitted later via
        av_last_group) to give the next phase's warm-up PE slack."""
        gs = list(range(nh * 4, nh * 4 + 4))
        # Z for all 4 groups of the half, packed as 4 columns of one bank.
        # One start=True zeroes the whole bank (2KB zero region); every
        # column then accumulates independently with start=False (first
        # touch of a pending-zero byte writes, later touches accumulate).
        zp = psz.tile([P, 4], f32, tag="z", name="zp")
        for mc in range(MC - 1):
            for gi, g in enumerate(gs):
                first = mc == 0 and gi == 0
                nc.tensor.matmul(
                    zp[:, gi:gi + 1], lhsT=et_sb[:, mc, g * P:(g + 1) * P],
                    rhs=ones_m, start=first, stop=first,
                    skip_group_check=not first,
                )
        uv = {}
        for g in gs[:3]:
            ur = ps.tile([P, 512], f32, tag="b", name="ur")
            ui = ps.tile([P, 512], f32, tag="b", name="ui")
            uv[g] = (ur, ui)
            for mc in range(MC - 1):
                lh = et_sb[:, mc, g * P:(g + 1) * P]
                nc.tensor.matmul(ur, lhsT=lh, rhs=v_sb[:, 0, mc, :],
                                 start=mc == 0, stop=False)
                nc.tensor.matmul(ui, lhsT=lh, rhs=v_sb[:, 1, mc, :],
                                 start=mc == 0, stop=False)
        for gi, g in enumerate(gs):
            nc.tensor.matmul(
                zp[:, gi:gi + 1], lhsT=et_sb[:, 7, g * P:(g + 1) * P],
                rhs=ones_m, start=False, stop=False,
                skip_group_check=True,
            )
        zr = tmp.tile([P, 4], f32, tag="zr", name="zr")
        nc.vector.reciprocal(zr, zp)
        av_zr[nh] = zr

        for gi, g in enumerate(gs[:3]):
            _av_close(g, gi, zr, *uv[g])
        if not defer_last_group:
            av_last_group(nh)

    def _av_close(g, gi, zr, ur, ui):
        lh = et_sb[:, 7, g * P:(g + 1) * P]
        nc.tensor.matmul(ur, lhsT=lh, rhs=v_sb[:, 0, 7, :],
                         start=False, stop=True)
        nc.tensor.matmul(ui, lhsT=lh, rhs=v_sb[:, 1, 7, :],
                         start=False, stop=True)
        # w = U * (1/Z); v bias added on the host (exact: sum(att) = 1).
        # re on DVE, im on ACT so the two tails run in parallel; one
        # combined DMA per group halves the shared-DMA-device overhead.
        o01 = outp.tile([P, 2, 512], f16, tag="o", name="o01")
        nc.vector.tensor_scalar_mul(o01[:, 0], ur, zr[:, gi:gi + 1])
        nc.scalar.activation(out=o01[:, 1], in_=ui, func=AF.Copy,
                             scale=zr[:, gi:gi + 1])
        nc.sync.dma_start(out=out_d[:, g * P:(g + 1) * P, :].rearrange(
            "t p d -> p t d"), in_=o01)

    def av_last_group(nh):
        g = nh * 4 + 3
        ur = ps.tile([P, 512], f32, tag="b", name="ur")
        ui = ps.tile([P, 512], f32, tag="b", name="ui")
        for mc in range(MC - 1):
            lh = et_sb[:, mc, g * P:(g + 1) * P]
            nc.tensor.matmul(ur, lhsT=lh, rhs=v_sb[:, 0, mc, :],
                             start=mc == 0, stop=False)
            nc.tensor.matmul(ui, lhsT=lh, rhs=v_sb[:, 1, mc, :],
                             start=mc == 0, stop=False)
        _av_close(g, 3, av_zr[nh], ur, ui)

    # First score half interleaved with the v projection (two v tiles held
    # back to cover part of the scores0 chain drain); each av half defers
    # its last m-chunks so score-tail chains drain under PE work.
    for mc in range(MC):
        score_tile(0, mc)
        if mc < 6:
            v_tile(mc)
    v_tile(6)
    v_tile(7)
    av_half(0, defer_last_group=True)
    score_tile(1, 0)
    av_last_group(0)          # av0's 4th group fills scores1's warm-up
    for mc in range(1, MC):
        score_tile(1, mc)
    av_half(1, defer_last_group=False)


def build_nc():
    nc = bacc.Bacc("TRN2", target_bir_lowering=False, debug=False)
    xr_d = nc.dram_tensor("xrT", [D, N], f16, kind="ExternalInput").ap()
    xi_d = nc.dram_tensor("xiT", [D, N], f16, kind="ExternalInput").ap()
    a_d = nc.dram_tensor("a6", [6, D, D], f16, kind="ExternalInput").ap()
    uq_d = nc.dram_tensor("uq", [1, 2, N], f16, kind="ExternalInput").ap()
    uk_d = nc.dram_tensor("uk", [P, MC, 2], f32, kind="ExternalInput").ap()
    out_d = nc.dram_tensor("out", [2, N, D], f16, kind="ExternalOutput").ap()
    with tile.TileContext(nc) as tc, ExitStack() as ctx:
        emit(tc, ctx, nc, xr_d, xi_d, a_d, uq_d, uk_d, out_d)
    nc.compile()
    return nc


def make_in_maps(inputs):
    sc = 1.0 / np.sqrt(D)

    def t16(a):
        return np.ascontiguousarray(a.T).astype(np.float16)

    Wq = inputs["Wq_re"].astype(np.float64) + 1j * inputs["Wq_im"].astype(np.float64)
    Wk = inputs["Wk_re"].astype(np.float64) + 1j * inputs["Wk_im"].astype(np.float64)
    bq = inputs["bq_re"].astype(np.float64) + 1j * inputs["bq_im"].astype(np.float64)
    bk = inputs["bk_re"].astype(np.float64) + 1j * inputs["bk_im"].astype(np.float64)

    A = (Wq.T @ Wk) * sc                       # [D, D] complex; lhsT layout
    Ar, Ai = A.real, A.imag
    a6 = np.stack([
        Ar.astype(np.float16), Ai.astype(np.float16),
        (Ar + Ai).astype(np.float16),
        t16(inputs["Wv_re"]), t16(inputs["Wv_im"]),
        t16(inputs["Wv_re"] + inputs["Wv_im"]),
    ])

    x = inputs["x_re"].astype(np.float64) + 1j * inputs["x_im"].astype(np.float64)
    aq = Wq.T @ bk                             # [D]
    bvec = Wk.T @ bq                           # [D]
    c00 = (bq * bk).sum()
    uqc = (x @ aq + c00) * sc                  # [B, N]
    ukc = (x @ bvec) * sc                      # [B, N]
    uq2 = np.stack([uqc.real, uqc.real + uqc.imag], axis=1).astype(np.float16)
    ukr = ukc.real.reshape(B, MC, P).transpose(0, 2, 1)
    uks = (2.0 * ukc.real + ukc.imag).reshape(B, MC, P).transpose(0, 2, 1)
    uk2 = np.stack([ukr, uks], axis=-1).astype(np.float32)  # [B, P, MC, 2]

    xrT = inputs["x_re"].transpose(0, 2, 1).astype(np.float16)  # [B, D, N]
    xiT = inputs["x_im"].transpose(0, 2, 1).astype(np.float16)
    return [
        {
            "xrT": np.ascontiguousarray(xrT[c]),
            "xiT": np.ascontiguousarray(xiT[c]),
            "a6": a6,
            "uq": np.ascontiguousarray(uq2[c])[None],
            "uk": np.ascontiguousarray(uk2[c]),
        }
        for c in range(NCORES)
    ]


_NC_CACHE = None


def get_nc():
    global _NC_CACHE
    if _NC_CACHE is None:
        _NC_CACHE = build_nc()
    return _NC_CACHE


def kernel(**inputs) -> np.ndarray:
    nc = get_nc()
    in_maps = make_in_maps(inputs)
    res = run_bass_kernel_spmd(nc, in_maps, core_ids=list(range(NCORES)))
    out = np.stack([res.results[c]["out"] for c in range(NCORES)], axis=1)
    out = out.astype(np.float32)
    out[0] += inputs["bv_re"].astype(np.float32)
    out[1] += inputs["bv_im"].astype(np.float32)
    return out
